# revision 38
# baseline (speedup 1.0000x reference)
"""Trainium2 Bass kernel for nn_CtrlPointHungarianMatcher.

Computes the DETR-style matching cost matrix
    C = 2*cost_class + 5*cost_kpts + 5*cost_mask_dice        (lvl < 2)
over pred (bs=8, Q=300) x tgt (T=160) with bilinear point-sampled dice cost
at P=12544 shared points.

Strategy: the bilinear gather of every mask at the P shared points is recast
as block-sparse matmuls (points sorted by mask row; per row-group a
host-built weight matrix A[x, point] multiplies the transposed mask row-slab
on the TensorEngine).  Sigmoid lands the sampled logits as sT[point, n],
feeding the dice numerator matmul against host-sampled target points.

Distribution: CORES NeuronCores each process BPC = bs/CORES batch elements
per dispatch (fewer cores than batch elements because per-dispatch client
overhead scales with core count in this runtime, while the per-core device
pipeline has slack).  Match-template tensors (A, tT, tsum, tgtbc — functions
of point_coords / tgt_pts / tgt_masks only) are baked into the NEFF as Const
tensors and loaded into SBUF ONCE per execution, shared by all BPC batches.
The jitted dispatch is AOT-compiled with bass fast-dispatch (no BassEffect)
so steady-state dispatch takes the C++ fast path.
"""

import hashlib
import numpy as np

_CACHE = {}

ALPHA = np.float32(0.25)
CLASS_W, COORD_W, MASK_W = 2.0, 5.0, 5.0
EPS = 1e-8

A_CHUNK = 2560      # A-matrix columns per DMA chunk
TPAD = 176          # padded t dim of tT (160 samples + 1 ones col + 15 zero;
                    # multiple of 16 so the DoubleRow k-tile stride is 16B-aligned)
CORES = 4           # NeuronCores used per dispatch
N_BS = 8            # batch elements (fixed by the problem)
ROWS_PER_DMA = 16   # mask pixel-rows per DMA chunk


# ----------------------------------------------------------------------------
# host-side point / weight preprocessing (float32-faithful to the reference)
# ----------------------------------------------------------------------------

def _point_grid(coords, H, W):
    c = coords.astype(np.float32)
    one = np.float32(1.0)
    gx = np.float32(2.0) * c[:, 0] - one
    gy = np.float32(2.0) * c[:, 1] - one
    x = ((gx + one) * np.float32(W) - one) * np.float32(0.5)
    y = ((gy + one) * np.float32(H) - one) * np.float32(0.5)
    x0f = np.floor(x)
    y0f = np.floor(y)
    wx1 = x - x0f
    wy1 = y - y0f
    wx0 = one - wx1
    wy0 = one - wy1
    return (x0f.astype(np.int32), y0f.astype(np.int32), wx0, wx1, wy0, wy1)


def _point_sample_np(masks, x0, y0, wx0, wx1, wy0, wy1, H, W):
    """Mirror of reference.point_sample for numpy float32 masks [R,H,W]."""
    x1 = x0 + 1
    y1 = y0 + 1

    def gather(ix, iy):
        valid = ((ix >= 0) & (ix < W) & (iy >= 0) & (iy < H)).astype(np.float32)
        ixc = np.clip(ix, 0, W - 1)
        iyc = np.clip(iy, 0, H - 1)
        return masks[:, iyc, ixc] * valid

    return (gather(x0, y0) * (wx0 * wy0)
            + gather(x1, y0) * (wx1 * wy0)
            + gather(x0, y1) * (wx0 * wy1)
            + gather(x1, y1) * (wx1 * wy1))


def _build_groups(x0, y0, wx0, wx1, wy0, wy1, H, W):
    """Sort points by y0; build per-row-group A tiles and metadata.

    Returns (perm, groups, A_packed, chunks) where groups is a list of dicts:
      p0, W: point range in sorted order
      tiles: list of (row, chunk_idx, local_col_ofs, ncols) - 1 or 2 entries
    """
    P = x0.shape[0]
    perm = np.argsort(y0, kind="stable")
    sx0, sy0 = x0[perm], y0[perm]
    swx0, swx1, swy0, swy1 = wx0[perm], wx1[perm], wy0[perm], wy1[perm]

    # group boundaries: distinct y0, split at 128 points
    groups_idx = []
    i = 0
    while i < P:
        j = i
        v = sy0[i]
        while j < P and sy0[j] == v and j - i < 128:
            j += 1
        groups_idx.append((i, j, int(v)))
        i = j

    # build A column blocks per group
    raw = []  # (row, cols[128, W] f32) per tile, in group order
    groups = []
    for (i, j, v) in groups_idx:
        Wg = j - i
        gx0 = sx0[i:j]
        gwx0, gwx1 = swx0[i:j], swx1[i:j]
        gwy0, gwy1 = swy0[i:j], swy1[i:j]
        has0 = 0 <= v < H
        has1 = 0 <= v + 1 < H
        tiles = []
        cols = np.arange(Wg)

        def make_tile(wy):
            A = np.zeros((128, Wg), np.float32)
            m0 = (gx0 >= 0) & (gx0 < W)
            if m0.any():
                A[gx0[m0], cols[m0]] += gwx0[m0] * wy[m0]
            x1 = gx0 + 1
            m1 = (x1 >= 0) & (x1 < W)
            if m1.any():
                A[x1[m1], cols[m1]] += gwx1[m1] * wy[m1]
            return A

        if has0:
            tiles.append([int(v), make_tile(gwy0)])
        if has1:
            tiles.append([int(v) + 1, make_tile(gwy1)])
        if not tiles:  # out-of-range row: contributes zero
            tiles.append([0, np.zeros((128, Wg), np.float32)])
        groups.append({"p0": i, "W": Wg, "tiles": tiles})
        raw.extend(tiles)

    # pack tiles into chunked A matrix, chunk boundaries on group boundaries.
    # Every tile starts on a 16-col boundary and occupies ceil16(W) cols so a
    # 2-tile group can be consumed by one DoubleRow matmul (k-tile stride must
    # be even and 16B-aligned).  first chunk is small so the pipeline starts
    # quickly.
    def ceil16(w):
        return (w + 15) // 16 * 16

    chunks = []  # list of ncols per chunk
    cur = 0
    for g in groups:
        gcols = sum(ceil16(t[1].shape[1]) for t in g["tiles"])
        limit = 800 if not chunks else A_CHUNK
        if cur + gcols > limit and cur > 0:
            chunks.append(cur)
            cur = 0
        for t in g["tiles"]:
            t.append(len(chunks))      # chunk idx
            t.append(cur)              # local col offset
            cur += ceil16(t[1].shape[1])
    if cur > 0:
        chunks.append(cur)

    total_cols = sum(chunks)
    A_packed = np.zeros((128, total_cols), np.float32)
    chunk_starts = np.cumsum([0] + chunks)
    for g in groups:
        for t in g["tiles"]:
            row, A, ci, ofs = t
            s = chunk_starts[ci] + ofs
            A_packed[:, s:s + A.shape[1]] = A

    return perm, groups, A_packed, chunks


# ----------------------------------------------------------------------------
# bass program builder
# ----------------------------------------------------------------------------

def _build_program(sig, shapes, consts, bpc):
    import concourse.bass as bass  # noqa: F401
    import concourse.tile as tile
    from concourse import mybir, bacc

    F32 = mybir.dt.float32
    F16 = mybir.dt.float16
    F8 = mybir.dt.float8e4
    AF = mybir.ActivationFunctionType
    ALU = mybir.AluOpType
    AX = mybir.AxisListType

    (n_rows, n_pix, CA, n_chunks_cols, group_meta, T, P, n_ctrl) = shapes
    H = W = int(np.sqrt(n_pix))
    KDIM = 2 * n_ctrl                      # 32
    TGTF = T * KDIM                        # 5120
    nbs = [(s, min(s + 128, n_rows)) for s in range(0, n_rows, 128)]
    n_nb = len(nbs)

    nc = bacc.Bacc("TRN2", target_bir_lowering=False, debug=False,
                   enable_partition_id=False)

    # per-call input (fp8), partition-major: one SLAB per batch element.
    # Within a slab, row p holds, for every mask pixel-row j, the 304-padded
    # transposed mask row [j*304 + q] = mask[q, j, p], followed by the packed
    # logits/points block (f16 values stored as byte pairs, bitcast on
    # device).  All device DMAs are contiguous per partition.
    LPW = n_nb * (1 + KDIM)
    n_rows_pix = n_pix // 128              # 128 pixel-rows
    MROW = n_rows_pix * 304
    SLAB = MROW + 2 * LPW
    # declared uint8 so the outer-jit HLO carries a standard dtype; the mask
    # bytes are bitcast to fp8 (and the logits/points block to f16) on device
    masks_ap = nc.dram_tensor("masksT", [128, bpc * SLAB], mybir.dt.uint8,
                              kind="ExternalInput").ap()
    C_ap = nc.dram_tensor("C", [bpc * n_rows, T], F32, kind="ExternalOutput").ap()

    # match-template constants baked into the NEFF
    A_ap = nc.inline_tensor(consts["A"], name="Ac").ap()
    tT_ap = nc.inline_tensor(consts["tT"], name="tTc").ap()
    tsum_ap = nc.inline_tensor(consts["tsum"], name="tsumc").ap()
    tgtbc_ap = nc.inline_tensor(consts["tgtbc"], name="tgtbcc").ap()

    n_groups_d = len(group_meta)
    n_pairs_d = (n_groups_d + 1) // 2

    with tile.TileContext(nc) as tc:
        import contextlib
        ctx = contextlib.ExitStack()
        with ctx:
            mt_pool = ctx.enter_context(tc.tile_pool(name="mt", bufs=3))
            a_pool = ctx.enter_context(tc.tile_pool(name="a", bufs=1))
            tt_pool = ctx.enter_context(tc.tile_pool(name="tt", bufs=1))
            st_pool = ctx.enter_context(tc.tile_pool(name="st", bufs=3))
            const_pool = ctx.enter_context(tc.tile_pool(name="const", bufs=1))
            fin_pool = ctx.enter_context(tc.tile_pool(name="fin", bufs=1))
            pu_pool = ctx.enter_context(tc.tile_pool(name="pu", bufs=3, space="PSUM"))
            nt_pool = ctx.enter_context(tc.tile_pool(name="nt", bufs=1, space="PSUM"))

            # constants
            ones1 = const_pool.tile([1, 128], F32, tag="ones1")
            nc.vector.memset(ones1[:], 1.0)
            eps_col = const_pool.tile([128, 1], F32, tag="epscol")
            nc.vector.memset(eps_col[:], float(EPS))

            tsm = const_pool.tile([1, T], F32, tag="tsm")
            nc.gpsimd.dma_start(tsm[:], tsum_ap[:])

            # t_sum broadcast [128, T]: matmul into a scratch pu buffer, then
            # park it in SBUF so PSUM keeps a bank free for a 3rd pu buffer
            # (deeper sampling->sigmoid pipelining)
            pden_ps = pu_pool.tile([128, 1024], F32, tag="pu", name="pden_ps")
            nc.tensor.matmul(pden_ps[:, 0:T], lhsT=ones1[:], rhs=tsm[:],
                             start=True, stop=True)
            pden = const_pool.tile([128, T], F32, tag="pden")
            nc.vector.tensor_scalar(out=pden[:], in0=pden_ps[:, 0:T],
                                    scalar1=0.0, scalar2=None, op0=ALU.add)

            # persistent numerator psums: nt01 holds nb0|nb1, nt2 holds nb2
            nt01 = nt_pool.tile([128, 512], F32, tag="nt01", name="nt01")
            nt2 = None
            if n_nb > 2:
                nt2 = nt_pool.tile([128, 256], F32, tag="nt2", name="nt2")

            def nt_slice(nb):
                if nb == 0:
                    return nt01[:, 0:256]
                if nb == 1:
                    return nt01[:, 256:512]
                return nt2[:, 0:256]

            # tT pair-batch tiles (fp8): resident for the whole execution
            PAIRS_PER_DMA = 4
            tt_tiles = [None] * ((n_pairs_d + PAIRS_PER_DMA - 1) // PAIRS_PER_DMA)

            def get_tt_batch(tb):
                if tt_tiles[tb] is None:
                    q0 = tb * PAIRS_PER_DMA
                    npr = min(PAIRS_PER_DMA, n_pairs_d - q0)
                    tt = tt_pool.tile([128, PAIRS_PER_DMA * 2 * TPAD], F8,
                                      tag=f"tt{tb}")
                    nc.gpsimd.dma_start(tt[:, 0:npr * 2 * TPAD],
                                        tT_ap[:, q0 * 2 * TPAD:(q0 + npr) * 2 * TPAD])
                    tt_tiles[tb] = tt
                return tt_tiles[tb]

            # A chunk tiles (fp8), resident for the whole execution
            a_tiles = [None] * len(n_chunks_cols)

            def get_a_chunk(ci):
                if a_tiles[ci] is None:
                    ncols = n_chunks_cols[ci]
                    cstart = sum(n_chunks_cols[:ci])
                    at = a_pool.tile([128, A_CHUNK], F8, tag=f"a{ci}")
                    nc.gpsimd.dma_start(at[:, 0:ncols], A_ap[:, cstart:cstart + ncols])
                    a_tiles[ci] = at
                return a_tiles[ci]

            # tgt control points broadcast [128, TGTF] (resident, f16)
            tgt_bc = fin_pool.tile([128, TGTF], F16, tag="tgtbc")

            def emit_tgt_chunk(i, n_chunks=4):
                c0 = i * (TGTF // n_chunks)
                c1 = (i + 1) * (TGTF // n_chunks)
                nc.gpsimd.dma_start(tgt_bc[:, c0:c1], tgtbc_ap[:, c0:c1])

            n_groups = len(group_meta)

            # ------------- per-batch state (reset each batch) -------------
            state = {}

            def emit_kpts(nb):
                b0, b1 = nbs[nb]
                nbW = b1 - b0
                ppk = state["ppk"]
                d_t = fin_pool.tile([128, TGTF], F16, tag="dkp", bufs=2)
                nc.vector.tensor_tensor(
                    out=d_t[0:nbW, :].rearrange("p (t k) -> p t k", k=KDIM),
                    in0=tgt_bc[0:nbW, :].rearrange("p (t k) -> p t k", k=KDIM),
                    in1=ppk[0:nbW, nb * KDIM:(nb + 1) * KDIM]
                        .broadcast_to([nbW, KDIM, T]).rearrange("p k t -> p t k"),
                    op=ALU.subtract)
                kp_t = fin_pool.tile([128, T], F32, tag=f"kp{nb}", bufs=2)
                nc.vector.tensor_reduce(
                    out=kp_t[0:nbW, :],
                    in_=d_t[0:nbW, :].rearrange("p (t k) -> p t k", k=KDIM),
                    axis=AX.X, op=ALU.add, apply_absolute_value=True)
                state[f"kp{nb}"] = kp_t

            # class-cost preludes for ALL batches, hoisted to program start and
            # grouped by activation function so the ACT table is switched a
            # constant number of times (steady state then runs Sigmoid only).
            prelude = {}

            def emit_preludes():
                cshape = [128, n_nb]

                def ptile(name, b):
                    return fin_pool.tile(cshape, F32, tag=f"{name}{b}",
                                         name=f"{name}{b}")

                for b in range(bpc):
                    lp16 = const_pool.tile([128, LPW], F16, tag=f"lp16_{b}",
                                           name=f"lp16_{b}")
                    src = masks_ap[:, b * SLAB + MROW:b * SLAB + MROW + 2 * LPW]
                    nc.gpsimd.dma_start(lp16[:], src.bitcast(F16))
                    prelude[f"lp16_{b}"] = lp16
                for b in range(bpc):
                    p_t = ptile("p", b)
                    nc.scalar.activation(p_t[:], prelude[f"lp16_{b}"][:, 0:n_nb],
                                         AF.Sigmoid)
                    prelude[f"p{b}"] = p_t
                for b in range(bpc):
                    q_t = ptile("q", b)
                    nc.vector.tensor_scalar(out=q_t[:], in0=prelude[f"p{b}"][:],
                                            scalar1=-1.0, scalar2=1.0,
                                            op0=ALU.mult, op1=ALU.add)
                    prelude[f"q{b}"] = q_t
                for b in range(bpc):
                    for nm in ("p", "q"):
                        ln_t = ptile(f"ln{nm}", b)
                        nc.scalar.activation(ln_t[:], prelude[f"{nm}{b}"][:],
                                             AF.Ln, bias=eps_col[:])
                        prelude[f"ln{nm}{b}"] = ln_t
                for b in range(bpc):
                    for nm in ("p", "q"):
                        sq_t = ptile(f"{nm}2", b)
                        nc.scalar.activation(sq_t[:], prelude[f"{nm}{b}"][:],
                                             AF.Square)
                        prelude[f"{nm}2{b}"] = sq_t
                for b in range(bpc):
                    t1_t = ptile("t1", b)
                    nc.vector.tensor_tensor(out=t1_t[:], in0=prelude[f"q2{b}"][:],
                                            in1=prelude[f"lnp{b}"][:], op=ALU.mult)
                    t2_t = ptile("t2", b)
                    nc.vector.tensor_tensor(out=t2_t[:], in0=prelude[f"p2{b}"][:],
                                            in1=prelude[f"lnq{b}"][:], op=ALU.mult)
                    t1s_t = ptile("t1s", b)
                    nc.vector.tensor_scalar(out=t1s_t[:], in0=t1_t[:],
                                            scalar1=-float(ALPHA) * CLASS_W,
                                            scalar2=5.0, op0=ALU.mult, op1=ALU.add)
                    base_t = ptile("base", b)
                    nc.vector.scalar_tensor_tensor(
                        out=base_t[:], in0=t2_t[:],
                        scalar=(1.0 - float(ALPHA)) * CLASS_W,
                        in1=t1s_t[:], op0=ALU.mult, op1=ALU.add)
                    prelude[f"base{b}"] = base_t

            pair_state = {}
            pending_nt = []

            def emit_pending_nt(keep=0):
                while len(pending_nt) > keep:
                    q, pu, st, tt, nsl = pending_nt.pop(0)
                    nc.scalar.activation(
                        st[:, 0:nsl * 304].rearrange("p (j n) -> p j n", n=304)[:, :, 0:n_rows],
                        pu[:, 0:nsl * 512].rearrange("p (j n) -> p j n", n=512)[:, :, 0:n_rows],
                        AF.Sigmoid)
                    tofs2 = (q % 4) * 2 * TPAD
                    st2 = st[:, 0:608].rearrange("p (two n) -> p two n", two=2)
                    tt2 = (tt[:, tofs2:tofs2 + 2 * TPAD]
                           .rearrange("p (two n) -> p two n", two=2))
                    if nsl == 2:
                        # DoubleRow: both groups of the pair as the 2 k-tiles
                        for nb, (b0, b1) in enumerate(nbs):
                            nc.tensor.matmul(
                                nt_slice(nb)[0:b1 - b0, 0:TPAD],
                                lhsT=st2[:, :, b0:b1],
                                rhs=tt2[:, :, 0:TPAD],
                                start=(q == 0),
                                stop=(2 * q + 1 == n_groups_d - 1),
                                perf_mode=mybir.MatmulPerfMode.DoubleRow,
                            )
                    else:
                        for nb, (b0, b1) in enumerate(nbs):
                            nc.tensor.matmul(
                                nt_slice(nb)[0:b1 - b0, 0:TPAD],
                                lhsT=st[:, b0:b1],
                                rhs=tt[:, tofs2:tofs2 + TPAD],
                                start=(q == 0),
                                stop=(2 * q == n_groups_d - 1),
                            )

            # mask pixel-rows r and r+1 are in the same DMA tile unless r+1
            # starts a new chunk
            bstarts = set([4] + list(range(16, n_rows_pix, ROWS_PER_DMA)))

            def emit_group(gi, mt_refs):
                p0, Wg, tiles = group_meta[gi]
                q, kk = gi // 2, gi % 2
                tb = q // 4
                tt = get_tt_batch(tb)
                if kk == 0:
                    pu = pu_pool.tile([128, 1024], F32, tag="pu")
                    st = st_pool.tile([128, 608], F8, tag="st")
                    pair_state[q] = (pu, st)
                else:
                    pu, st = pair_state.pop(q)
                pofs = kk * 512
                ci0 = tiles[0][1]
                at = get_a_chunk(ci0)
                if ci0 + 1 < len(n_chunks_cols):
                    get_a_chunk(ci0 + 1)  # prefetch
                wpad = (Wg + 15) // 16 * 16
                use_dr = (len(tiles) == 2
                          and tiles[1][0] == tiles[0][0] + 1
                          and tiles[1][2] == tiles[0][2] + wpad
                          and tiles[1][1] == ci0
                          and tiles[1][0] not in bstarts)
                if use_dr:
                    row, _, ofs, _ = tiles[0]
                    mt, j = mt_refs[row]
                    a2 = (at[:, ofs:ofs + 2 * wpad]
                          .rearrange("p (two w) -> p two w", two=2)[:, :, 0:Wg])
                    m2 = (mt[:, j * 304:(j + 2) * 304]
                          .rearrange("p (two n) -> p two n", two=2))
                    nc.tensor.matmul(
                        pu[0:Wg, pofs:pofs + n_rows],
                        lhsT=a2,
                        rhs=m2[:, :, 0:n_rows],
                        start=True, stop=True,
                        perf_mode=mybir.MatmulPerfMode.DoubleRow,
                    )
                else:
                    n_t = len(tiles)
                    for k, (row, ci, ofs, ncols) in enumerate(tiles):
                        at_k = get_a_chunk(ci)
                        mt, j = mt_refs[row]
                        nc.tensor.matmul(
                            pu[0:Wg, pofs:pofs + n_rows],
                            lhsT=at_k[:, ofs:ofs + ncols],
                            rhs=mt[:, j * 304:j * 304 + n_rows],
                            start=(k == 0),
                            stop=(k == n_t - 1),
                        )
                lone = (kk == 0 and gi == n_groups_d - 1)
                if kk == 1 or lone:
                    pending_nt.append((q, pu, st, tt, 1 if lone else 2))
                    emit_pending_nt(keep=1)

            def finalize_batch(b):
                for nb, (b0, b1) in enumerate(nbs):
                    nbW = b1 - b0
                    nts = nt_slice(nb)
                    # s_sum + 1  (on DVE so the ACT engine stays on Sigmoid)
                    ssum1 = fin_pool.tile([128, 1], F32, tag=f"ssum{nb}")
                    nc.vector.tensor_scalar(out=ssum1[0:nbW, :],
                                            in0=nts[0:nbW, 160:161],
                                            scalar1=1.0, scalar2=None,
                                            op0=ALU.add)
                    kp_t = state[f"kp{nb}"]
                    # den1 = t_sum_bcast + (s_sum + 1)
                    den1 = fin_pool.tile([128, T], F32, tag=f"den{nb}")
                    nc.vector.tensor_scalar(out=den1[0:nbW, :], in0=pden[0:nbW, 0:T],
                                            scalar1=ssum1[0:nbW, 0:1], scalar2=None,
                                            op0=ALU.add)
                    inv = fin_pool.tile([128, T], F32, tag=f"inv{nb}")
                    nc.vector.reciprocal(inv[0:nbW, :], den1[0:nbW, :])
                    # w = (2*num + 1) * inv
                    q2n = fin_pool.tile([128, T], F32, tag=f"q2n{nb}")
                    nc.vector.tensor_scalar(out=q2n[0:nbW, :], in0=nts[0:nbW, 0:T],
                                            scalar1=2.0, scalar2=1.0,
                                            op0=ALU.mult, op1=ALU.add)
                    w_t = fin_pool.tile([128, T], F32, tag=f"w{nb}")
                    nc.vector.tensor_tensor(out=w_t[0:nbW, :], in0=q2n[0:nbW, :],
                                            in1=inv[0:nbW, :], op=ALU.mult)
                    # C = 5*(kpts - w) + base
                    km_t = fin_pool.tile([128, T], F32, tag=f"km{nb}")
                    nc.vector.tensor_tensor(out=km_t[0:nbW, :], in0=kp_t[0:nbW, :],
                                            in1=w_t[0:nbW, :], op=ALU.subtract)
                    c_t = fin_pool.tile([128, T], F32, tag=f"c{nb}", bufs=2)
                    nc.vector.tensor_scalar(out=c_t[0:nbW, :], in0=km_t[0:nbW, :],
                                            scalar1=float(COORD_W),
                                            scalar2=prelude[f"base{b}"][0:nbW, nb:nb + 1],
                                            op0=ALU.mult, op1=ALU.add)
                    nc.sync.dma_start(C_ap[b * n_rows + b0:b * n_rows + b1, :],
                                      c_t[0:nbW, 0:T])

            # ---------------- per-batch mask pipeline ----------------
            # zero both pu rotation buffers once: PSUM holds stale data from
            # whatever ran before (possibly NaN), and partitions beyond a
            # group's Wg flow through sigmoid into the numerator matmul where
            # 0*NaN would poison the accumulation.  After this, every value
            # that ever lands in pu is finite.
            for _ in range(3):
                pz = pu_pool.tile([128, 1024], F32, tag="pu", name="pz")
                nc.vector.memset(pz[:], 0.0)
            get_a_chunk(0)
            get_tt_batch(0)
            emit_preludes()
            NTC = 8
            for b in range(bpc):
                mt_refs = {}
                g_idx = 0
                mofs = b * SLAB
                state["ppk"] = prelude[f"lp16_{b}"][:, n_nb:LPW]
                batch_starts = [0, 4] + list(range(16, n_rows_pix, ROWS_PER_DMA))
                if b == 0:
                    stage_at = {
                        16: lambda: (emit_tgt_chunk(0, NTC),
                                     emit_tgt_chunk(1, NTC), emit_tgt_chunk(2, NTC)),
                        32: lambda: (emit_tgt_chunk(3, NTC), emit_tgt_chunk(4, NTC),
                                     emit_tgt_chunk(5, NTC)),
                        48: lambda: (emit_tgt_chunk(6, NTC), emit_tgt_chunk(7, NTC)),
                        64: lambda: (emit_kpts(0), emit_kpts(1), emit_kpts(2)),
                    }
                else:
                    stage_at = {
                        64: lambda: (emit_kpts(0), emit_kpts(1), emit_kpts(2)),
                    }
                for bi, rb in enumerate(batch_starts):
                    fn = stage_at.get(rb)
                    if fn is not None:
                        fn()
                    nxt = batch_starts[bi + 1] if bi + 1 < len(batch_starts) else n_rows_pix
                    nr = nxt - rb
                    mt = mt_pool.tile([128, ROWS_PER_DMA * 304], F8, tag="mt")
                    nc.sync.dma_start(
                        mt[:, 0:nr * 304],
                        masks_ap[:, mofs + rb * 304:mofs + (rb + nr) * 304]
                        .bitcast(F8))
                    for j in range(nr):
                        r = rb + j
                        mt_refs[r] = (mt, j)
                        while g_idx < n_groups and group_meta[g_idx][2][-1][0] <= r:
                            emit_group(g_idx, mt_refs)
                            g_idx += 1
                while g_idx < n_groups:
                    emit_group(g_idx, mt_refs)
                    g_idx += 1
                emit_pending_nt()
                finalize_batch(b)

    nc.compile()
    return nc


def _get_runner(sig, shapes, consts, bpc, cores):
    ent = _CACHE.get(sig)
    if ent is not None:
        return ent
    import jax
    from jax.sharding import Mesh, PartitionSpec, NamedSharding
    from concourse import mybir
    from concourse.bass2jax import _bass_exec_p, install_neuronx_cc_hook

    install_neuronx_cc_hook()
    nc = _build_program(sig, shapes, consts, bpc)

    in_names, out_names, out_avals, zero_outs = [], [], [], []
    in_shapes = {}
    for alloc in nc.m.functions[0].allocations:
        if not isinstance(alloc, mybir.MemoryLocationSet):
            continue
        name = alloc.memorylocations[0].name
        if alloc.kind == "ExternalInput":
            in_names.append(name)
            in_shapes[name] = (tuple(alloc.tensor_shape), mybir.dt.np(alloc.dtype))
        elif alloc.kind == "ExternalOutput":
            shape = tuple(alloc.tensor_shape)
            dtype = mybir.dt.np(alloc.dtype)
            out_names.append(name)
            out_avals.append(jax.core.ShapedArray(shape, dtype))
            zero_outs.append(np.zeros(shape, dtype))

    all_in = tuple(in_names) + tuple(out_names)

    def _body(*args):
        return tuple(_bass_exec_p.bind(
            *args, out_avals=tuple(out_avals), in_names=all_in,
            out_names=tuple(out_names), lowering_input_output_aliases=(),
            sim_require_finite=False, sim_require_nnan=False, nc=nc))

    devs = jax.devices()[:cores]
    mesh = Mesh(np.asarray(devs), ("core",))
    nargs = len(in_names) + len(out_names)
    sm_kwargs = dict(mesh=mesh,
                     in_specs=(PartitionSpec("core"),) * nargs,
                     out_specs=(PartitionSpec("core"),) * len(out_names))
    sharding = NamedSharding(mesh, PartitionSpec("core"))

    def _make_shard_mapped():
        try:
            from jax import shard_map as _sm  # jax >= 0.8
            return _sm(_body, check_vma=False, **sm_kwargs)
        except (ImportError, TypeError):
            from jax.experimental.shard_map import shard_map as _sm_old
            return _sm_old(_body, check_rep=False, **sm_kwargs)

    # global (pre-shard_map) avals for AOT lowering
    shaped = []
    for name in in_names:
        shp, dt = in_shapes[name]
        shaped.append(jax.ShapeDtypeStruct((cores * shp[0],) + shp[1:], dt,
                                           sharding=sharding))
    for z in zero_outs:
        shaped.append(jax.ShapeDtypeStruct((cores * z.shape[0],) + z.shape[1:],
                                           z.dtype, sharding=sharding))

    sharded = None
    try:
        from concourse.bass2jax import fast_dispatch_compile

        def compile_fn():
            jitted = jax.jit(_make_shard_mapped(), keep_unused=True)
            return jitted.lower(*shaped).compile()

        sharded = fast_dispatch_compile(compile_fn)
        try:
            # drop the per-call safety-net shard walk (pure-python overhead on
            # every dispatch); our callers always read the outputs, so device
            # errors still surface at block_until_ready
            import jax._src.stages as _jstages
            sharded.__class__ = _jstages.Compiled
        except Exception:
            pass
    except Exception:
        import traceback
        traceback.print_exc()
        sharded = jax.jit(_make_shard_mapped(), keep_unused=True)

    ent = (sharded, in_names, out_names, zero_outs, sharding)
    _CACHE[sig] = ent
    return ent


# ----------------------------------------------------------------------------
# host fallback (lvl >= 2, or if the device path is unavailable)
# ----------------------------------------------------------------------------

def _host_reference(pred_logits, pred_ctrl_points, pred_seg_mask, tgt_pts,
                    tgt_masks, point_coords, lvl):
    bs, Q = pred_logits.shape[:2]
    N = bs * Q
    p = 1.0 / (1.0 + np.exp(-pred_logits.reshape(N, -1).astype(np.float64)))
    out_pts = pred_ctrl_points.reshape(N, -1).astype(np.float64)
    tgt_flat = tgt_pts.reshape(tgt_pts.shape[0], -1).astype(np.float64)

    cost_mask_dice = 0.0
    if int(lvl) < 2:
        H, W = pred_seg_mask.shape[-2:]
        x0, y0, wx0, wx1, wy0, wy1 = _point_grid(point_coords, H, W)
        t_samp = _point_sample_np(tgt_masks.astype(np.float32), x0, y0,
                                  wx0, wx1, wy0, wy1, H, W).astype(np.float64)
        o_masks = pred_seg_mask.reshape(N, H, W).astype(np.float32)
        o_samp = _point_sample_np(o_masks, x0, y0, wx0, wx1, wy0, wy1, H, W)
        s = 1.0 / (1.0 + np.exp(-o_samp.astype(np.float64)))
        num = 2.0 * (s @ t_samp.T)
        den = s.sum(-1)[:, None] + t_samp.sum(-1)[None, :]
        cost_mask_dice = 1.0 - (num + 1.0) / (den + 1.0)

    neg = (1 - 0.25) * p ** 2.0 * (-np.log(1.0 - p + EPS))
    pos = 0.25 * (1.0 - p) ** 2.0 * (-np.log(p + EPS))
    cost_class = pos - neg
    cost_kpts = np.abs(out_pts[:, None, :] - tgt_flat[None, :, :]).sum(-1)
    C = CLASS_W * cost_class + COORD_W * cost_kpts + MASK_W * cost_mask_dice
    return C.reshape(bs, Q, -1).astype(np.float32)


# ----------------------------------------------------------------------------
# main entry
# ----------------------------------------------------------------------------

_PREP_CACHE = {}


def _prepare(pred_logits, pred_ctrl_points, pred_seg_mask, tgt_pts, tgt_masks,
             point_coords):
    import ml_dtypes

    bs, Q = pred_logits.shape[:2]
    H, W = pred_seg_mask.shape[-2:]
    T = tgt_masks.shape[0]
    P = point_coords.shape[0]
    n_ctrl = pred_ctrl_points.shape[2]
    KDIM = 2 * n_ctrl

    key = hashlib.sha1()
    key.update(np.ascontiguousarray(point_coords).tobytes())
    key.update(np.ascontiguousarray(tgt_pts).tobytes())
    key.update(np.ascontiguousarray(tgt_masks).tobytes())
    key.update(str((bs, Q, H, W, T, P, n_ctrl, CORES)).encode())
    pkey = key.hexdigest()
    ent = _PREP_CACHE.get(pkey)
    if ent is not None:
        return ent

    x0, y0, wx0, wx1, wy0, wy1 = _point_grid(point_coords, H, W)
    perm, groups, A_packed, chunks = _build_groups(x0, y0, wx0, wx1, wy0, wy1, H, W)

    # target-side samples at the same points, in sorted-point order
    t_samp = _point_sample_np(tgt_masks.astype(np.float32), x0, y0,
                              wx0, wx1, wy0, wy1, H, W)      # [T, P] f32
    t_sum = t_samp.sum(axis=1, dtype=np.float32)             # [T]
    tTs = t_samp.T[perm].astype(np.float16)                  # [P, T] sorted
    # pair-interleaved packing: pair q = groups (2q, 2q+1); 128-row block q
    # holds [groupA TPAD cols | groupB TPAD cols] so DMA runs are 2*TPAD wide
    n_groups = len(groups)
    n_pairs = (n_groups + 1) // 2
    tT = np.zeros((n_pairs * 128, 2 * TPAD), np.float16)
    for gi, g in enumerate(groups):
        q, k = gi // 2, gi % 2
        r0 = q * 128
        sl = tT[r0:r0 + g["W"], k * TPAD:k * TPAD + T]
        sl[:] = tTs[g["p0"]:g["p0"] + g["W"]]
        tT[r0:r0 + g["W"], k * TPAD + T] = 1.0  # ones col -> s row-sum
    # partition-major repack: row p holds pair q's 2*TPAD block at q*2*TPAD
    tT = np.ascontiguousarray(
        tT.reshape(n_pairs, 128, 2 * TPAD).transpose(1, 0, 2)
          .reshape(128, n_pairs * 2 * TPAD))

    tgt_bc = np.ascontiguousarray(
        np.broadcast_to(tgt_pts.reshape(1, T * KDIM).astype(np.float16),
                        (128, T * KDIM)))

    consts = {
        "A": A_packed.astype(ml_dtypes.float8_e4m3),
        "tT": tT.astype(ml_dtypes.float8_e4m3),
        "tsum": t_sum.reshape(1, T).astype(np.float32),
        "tgtbc": tgt_bc,
    }

    group_meta = tuple(
        (g["p0"], g["W"], tuple((t[0], t[2], t[3], t[1].shape[1]) for t in g["tiles"]))
        for g in groups
    )
    sig = (bs, Q, H, W, T, P, n_ctrl, tuple(chunks), pkey)
    shapes = (Q, H * W, A_packed.shape[1], tuple(chunks), group_meta, T, P, n_ctrl)
    ent = (sig, shapes, consts)
    _PREP_CACHE[pkey] = ent
    return ent


def _make_in_maps(pred_logits, pred_ctrl_points, pred_seg_mask, Q, n_ctrl):
    """Per-core input dicts: masksT (fp8) in partition-major layout —
    masksT[x, b*SLAB + y*304 + q] = mask[b, q, y, x] for each of the core's
    BPC batch elements b, each slab followed by the packed logits|pts block
    (f16 values as byte pairs, bitcast on device)."""
    import ml_dtypes

    bs = pred_logits.shape[0]
    bpc = bs // CORES
    KDIM = 2 * n_ctrl
    n_nb = (Q + 127) // 128
    Hm, Wm = pred_seg_mask.shape[-2:]
    LPW = n_nb * (1 + KDIM)
    MROW = Hm * 304
    SLAB = MROW + 2 * LPW
    in_maps = []
    for c in range(CORES):
        mext8 = np.zeros((Wm, bpc * SLAB), ml_dtypes.float8_e4m3)
        mext = mext8.view(np.uint8)
        for j in range(bpc):
            b = c * bpc + j
            lp = np.zeros((128, LPW), np.float32)
            lg = pred_logits[b].reshape(Q)
            pc = pred_ctrl_points[b].reshape(Q, KDIM)
            for nb in range(n_nb):
                b0, b1 = nb * 128, min(nb * 128 + 128, Q)
                lp[0:b1 - b0, nb] = lg[b0:b1]
                lp[0:b1 - b0, n_nb + nb * KDIM:n_nb + (nb + 1) * KDIM] = pc[b0:b1]
            m3 = np.zeros((Wm, Hm, 304), ml_dtypes.float8_e4m3)
            m3[:, :, 0:Q] = pred_seg_mask[b].transpose(2, 1, 0).astype(
                ml_dtypes.float8_e4m3)
            mext8[:, j * SLAB:j * SLAB + MROW] = m3.reshape(Wm, MROW)
            mext[:, j * SLAB + MROW:(j + 1) * SLAB] = (
                lp.astype(np.float16).view(np.uint8))
        in_maps.append({"masksT": mext})
    return in_maps


def kernel(pred_logits, pred_ctrl_points, pred_seg_mask, tgt_pts, tgt_masks,
           point_coords, lvl):
    pred_logits = np.asarray(pred_logits)
    pred_ctrl_points = np.asarray(pred_ctrl_points)
    pred_seg_mask = np.asarray(pred_seg_mask)
    tgt_pts = np.asarray(tgt_pts)
    tgt_masks = np.asarray(tgt_masks)
    point_coords = np.asarray(point_coords)

    if int(lvl) >= 2:
        return _host_reference(pred_logits, pred_ctrl_points, pred_seg_mask,
                               tgt_pts, tgt_masks, point_coords, lvl)

    try:
        return _device_kernel(pred_logits, pred_ctrl_points, pred_seg_mask,
                              tgt_pts, tgt_masks, point_coords)
    except Exception:
        import traceback
        traceback.print_exc()
        return _host_reference(pred_logits, pred_ctrl_points, pred_seg_mask,
                               tgt_pts, tgt_masks, point_coords, lvl)


def _device_kernel(pred_logits, pred_ctrl_points, pred_seg_mask, tgt_pts,
                   tgt_masks, point_coords):
    import jax

    bs, Q = pred_logits.shape[:2]
    T = tgt_masks.shape[0]
    n_ctrl = pred_ctrl_points.shape[2]

    sig, shapes, consts = _prepare(pred_logits, pred_ctrl_points, pred_seg_mask,
                                   tgt_pts, tgt_masks, point_coords)
    if bs % CORES != 0 or len(jax.devices()) < CORES:
        raise RuntimeError(f"need bs % {CORES} == 0 cores, got bs={bs}, "
                           f"{len(jax.devices())} devices")
    bpc = bs // CORES
    sharded, in_names, out_names, zero_outs, sharding = _get_runner(
        sig, shapes, consts, bpc, CORES)
    in_maps = _make_in_maps(pred_logits, pred_ctrl_points, pred_seg_mask,
                            Q, n_ctrl)

    gargs = [
        jax.device_put(
            np.concatenate([in_maps[c][n] for c in range(CORES)], axis=0), sharding)
        for n in in_names
    ]
    gargs += [
        jax.device_put(
            np.zeros((CORES * z.shape[0], *z.shape[1:]), z.dtype), sharding)
        for z in zero_outs
    ]
    outs = sharded(*gargs)
    C = np.asarray(outs[0]).reshape(bs, Q, T)
    return C.astype(np.float32)


# revision 39
# speedup vs baseline: 1.0030x; 1.0030x over previous
"""Trainium2 Bass kernel for nn_CtrlPointHungarianMatcher.

Computes the DETR-style matching cost matrix
    C = 2*cost_class + 5*cost_kpts + 5*cost_mask_dice        (lvl < 2)
over pred (bs=8, Q=300) x tgt (T=160) with bilinear point-sampled dice cost
at P=12544 shared points.

Strategy: the bilinear gather of every mask at the P shared points is recast
as block-sparse matmuls (points sorted by mask row; per row-group a
host-built weight matrix A[x, point] multiplies the transposed mask row-slab
on the TensorEngine).  Sigmoid lands the sampled logits as sT[point, n],
feeding the dice numerator matmul against host-sampled target points.

Distribution: CORES NeuronCores each process BPC = bs/CORES batch elements
per dispatch (fewer cores than batch elements because per-dispatch client
overhead scales with core count in this runtime, while the per-core device
pipeline has slack).  Match-template tensors (A, tT, tsum, tgtbc — functions
of point_coords / tgt_pts / tgt_masks only) are baked into the NEFF as Const
tensors and loaded into SBUF ONCE per execution, shared by all BPC batches.
The jitted dispatch is AOT-compiled with bass fast-dispatch (no BassEffect)
so steady-state dispatch takes the C++ fast path.
"""

import hashlib
import numpy as np

_CACHE = {}

ALPHA = np.float32(0.25)
CLASS_W, COORD_W, MASK_W = 2.0, 5.0, 5.0
EPS = 1e-8

A_CHUNK = 2560      # A-matrix columns per DMA chunk
TPAD = 176          # padded t dim of tT (160 samples + 1 ones col + 15 zero;
                    # multiple of 16 so the DoubleRow k-tile stride is 16B-aligned)
CORES = 4           # NeuronCores used per dispatch
N_BS = 8            # batch elements (fixed by the problem)
ROWS_PER_DMA = 16   # mask pixel-rows per DMA chunk


# ----------------------------------------------------------------------------
# host-side point / weight preprocessing (float32-faithful to the reference)
# ----------------------------------------------------------------------------

def _point_grid(coords, H, W):
    c = coords.astype(np.float32)
    one = np.float32(1.0)
    gx = np.float32(2.0) * c[:, 0] - one
    gy = np.float32(2.0) * c[:, 1] - one
    x = ((gx + one) * np.float32(W) - one) * np.float32(0.5)
    y = ((gy + one) * np.float32(H) - one) * np.float32(0.5)
    x0f = np.floor(x)
    y0f = np.floor(y)
    wx1 = x - x0f
    wy1 = y - y0f
    wx0 = one - wx1
    wy0 = one - wy1
    return (x0f.astype(np.int32), y0f.astype(np.int32), wx0, wx1, wy0, wy1)


def _point_sample_np(masks, x0, y0, wx0, wx1, wy0, wy1, H, W):
    """Mirror of reference.point_sample for numpy float32 masks [R,H,W]."""
    x1 = x0 + 1
    y1 = y0 + 1

    def gather(ix, iy):
        valid = ((ix >= 0) & (ix < W) & (iy >= 0) & (iy < H)).astype(np.float32)
        ixc = np.clip(ix, 0, W - 1)
        iyc = np.clip(iy, 0, H - 1)
        return masks[:, iyc, ixc] * valid

    return (gather(x0, y0) * (wx0 * wy0)
            + gather(x1, y0) * (wx1 * wy0)
            + gather(x0, y1) * (wx0 * wy1)
            + gather(x1, y1) * (wx1 * wy1))


def _build_groups(x0, y0, wx0, wx1, wy0, wy1, H, W):
    """Sort points by y0; build per-row-group A tiles and metadata.

    Returns (perm, groups, A_packed, chunks) where groups is a list of dicts:
      p0, W: point range in sorted order
      tiles: list of (row, chunk_idx, local_col_ofs, ncols) - 1 or 2 entries
    """
    P = x0.shape[0]
    perm = np.argsort(y0, kind="stable")
    sx0, sy0 = x0[perm], y0[perm]
    swx0, swx1, swy0, swy1 = wx0[perm], wx1[perm], wy0[perm], wy1[perm]

    # group boundaries: distinct y0, split at 128 points
    groups_idx = []
    i = 0
    while i < P:
        j = i
        v = sy0[i]
        while j < P and sy0[j] == v and j - i < 128:
            j += 1
        groups_idx.append((i, j, int(v)))
        i = j

    # build A column blocks per group
    raw = []  # (row, cols[128, W] f32) per tile, in group order
    groups = []
    for (i, j, v) in groups_idx:
        Wg = j - i
        gx0 = sx0[i:j]
        gwx0, gwx1 = swx0[i:j], swx1[i:j]
        gwy0, gwy1 = swy0[i:j], swy1[i:j]
        has0 = 0 <= v < H
        has1 = 0 <= v + 1 < H
        tiles = []
        cols = np.arange(Wg)

        def make_tile(wy):
            A = np.zeros((128, Wg), np.float32)
            m0 = (gx0 >= 0) & (gx0 < W)
            if m0.any():
                A[gx0[m0], cols[m0]] += gwx0[m0] * wy[m0]
            x1 = gx0 + 1
            m1 = (x1 >= 0) & (x1 < W)
            if m1.any():
                A[x1[m1], cols[m1]] += gwx1[m1] * wy[m1]
            return A

        if has0:
            tiles.append([int(v), make_tile(gwy0)])
        if has1:
            tiles.append([int(v) + 1, make_tile(gwy1)])
        if not tiles:  # out-of-range row: contributes zero
            tiles.append([0, np.zeros((128, Wg), np.float32)])
        groups.append({"p0": i, "W": Wg, "tiles": tiles})
        raw.extend(tiles)

    # pack tiles into chunked A matrix, chunk boundaries on group boundaries.
    # Every tile starts on a 16-col boundary and occupies ceil16(W) cols so a
    # 2-tile group can be consumed by one DoubleRow matmul (k-tile stride must
    # be even and 16B-aligned).  first chunk is small so the pipeline starts
    # quickly.
    def ceil16(w):
        return (w + 15) // 16 * 16

    chunks = []  # list of ncols per chunk
    cur = 0
    for g in groups:
        gcols = sum(ceil16(t[1].shape[1]) for t in g["tiles"])
        limit = 800 if not chunks else A_CHUNK
        if cur + gcols > limit and cur > 0:
            chunks.append(cur)
            cur = 0
        for t in g["tiles"]:
            t.append(len(chunks))      # chunk idx
            t.append(cur)              # local col offset
            cur += ceil16(t[1].shape[1])
    if cur > 0:
        chunks.append(cur)

    total_cols = sum(chunks)
    A_packed = np.zeros((128, total_cols), np.float32)
    chunk_starts = np.cumsum([0] + chunks)
    for g in groups:
        for t in g["tiles"]:
            row, A, ci, ofs = t
            s = chunk_starts[ci] + ofs
            A_packed[:, s:s + A.shape[1]] = A

    return perm, groups, A_packed, chunks


# ----------------------------------------------------------------------------
# bass program builder
# ----------------------------------------------------------------------------

def _build_program(sig, shapes, consts, bpc):
    import concourse.bass as bass  # noqa: F401
    import concourse.tile as tile
    from concourse import mybir, bacc

    F32 = mybir.dt.float32
    F16 = mybir.dt.float16
    F8 = mybir.dt.float8e4
    AF = mybir.ActivationFunctionType
    ALU = mybir.AluOpType
    AX = mybir.AxisListType

    (n_rows, n_pix, CA, n_chunks_cols, group_meta, T, P, n_ctrl) = shapes
    H = W = int(np.sqrt(n_pix))
    KDIM = 2 * n_ctrl                      # 32
    TGTF = T * KDIM                        # 5120
    nbs = [(s, min(s + 128, n_rows)) for s in range(0, n_rows, 128)]
    n_nb = len(nbs)

    nc = bacc.Bacc("TRN2", target_bir_lowering=False, debug=False,
                   enable_partition_id=False)

    # per-call input (fp8), partition-major: one SLAB per batch element.
    # Within a slab, row p holds, for every mask pixel-row j, the 304-padded
    # transposed mask row [j*304 + q] = mask[q, j, p], followed by the packed
    # logits/points block (f16 values stored as byte pairs, bitcast on
    # device).  All device DMAs are contiguous per partition.
    LPW = n_nb * (1 + KDIM)
    n_rows_pix = n_pix // 128              # 128 pixel-rows
    MROW = n_rows_pix * 304
    SLAB = MROW + 2 * LPW
    # declared uint8 so the outer-jit HLO carries a standard dtype; the mask
    # bytes are bitcast to fp8 (and the logits/points block to f16) on device
    masks_ap = nc.dram_tensor("masksT", [128, bpc * SLAB], mybir.dt.uint8,
                              kind="ExternalInput").ap()
    C_ap = nc.dram_tensor("C", [bpc * n_rows, T], F32, kind="ExternalOutput").ap()

    # match-template constants baked into the NEFF
    A_ap = nc.inline_tensor(consts["A"], name="Ac").ap()
    tT_ap = nc.inline_tensor(consts["tT"], name="tTc").ap()
    tsum_ap = nc.inline_tensor(consts["tsum"], name="tsumc").ap()
    tgtbc_ap = nc.inline_tensor(consts["tgtbc"], name="tgtbcc").ap()

    n_groups_d = len(group_meta)
    n_pairs_d = (n_groups_d + 1) // 2

    with tile.TileContext(nc) as tc:
        import contextlib
        ctx = contextlib.ExitStack()
        with ctx:
            mt_pool = ctx.enter_context(tc.tile_pool(name="mt", bufs=3))
            a_pool = ctx.enter_context(tc.tile_pool(name="a", bufs=1))
            tt_pool = ctx.enter_context(tc.tile_pool(name="tt", bufs=1))
            st_pool = ctx.enter_context(tc.tile_pool(name="st", bufs=4))
            const_pool = ctx.enter_context(tc.tile_pool(name="const", bufs=1))
            fin_pool = ctx.enter_context(tc.tile_pool(name="fin", bufs=1))
            pu_pool = ctx.enter_context(tc.tile_pool(name="pu", bufs=3, space="PSUM"))
            nt_pool = ctx.enter_context(tc.tile_pool(name="nt", bufs=1, space="PSUM"))

            # constants
            ones1 = const_pool.tile([1, 128], F32, tag="ones1")
            nc.vector.memset(ones1[:], 1.0)
            eps_col = const_pool.tile([128, 1], F32, tag="epscol")
            nc.vector.memset(eps_col[:], float(EPS))

            tsm = const_pool.tile([1, T], F32, tag="tsm")
            nc.gpsimd.dma_start(tsm[:], tsum_ap[:])

            # t_sum broadcast [128, T]: matmul into a scratch pu buffer, then
            # park it in SBUF so PSUM keeps a bank free for a 3rd pu buffer
            # (deeper sampling->sigmoid pipelining)
            pden_ps = pu_pool.tile([128, 1024], F32, tag="pu", name="pden_ps")
            nc.tensor.matmul(pden_ps[:, 0:T], lhsT=ones1[:], rhs=tsm[:],
                             start=True, stop=True)
            pden = const_pool.tile([128, T], F32, tag="pden")
            nc.vector.tensor_scalar(out=pden[:], in0=pden_ps[:, 0:T],
                                    scalar1=0.0, scalar2=None, op0=ALU.add)

            # persistent numerator psums: nt01 holds nb0|nb1, nt2 holds nb2
            nt01 = nt_pool.tile([128, 512], F32, tag="nt01", name="nt01")
            nt2 = None
            if n_nb > 2:
                nt2 = nt_pool.tile([128, 256], F32, tag="nt2", name="nt2")

            def nt_slice(nb):
                if nb == 0:
                    return nt01[:, 0:256]
                if nb == 1:
                    return nt01[:, 256:512]
                return nt2[:, 0:256]

            # tT pair-batch tiles (fp8): resident for the whole execution
            PAIRS_PER_DMA = 4
            tt_tiles = [None] * ((n_pairs_d + PAIRS_PER_DMA - 1) // PAIRS_PER_DMA)

            def get_tt_batch(tb):
                if tt_tiles[tb] is None:
                    q0 = tb * PAIRS_PER_DMA
                    npr = min(PAIRS_PER_DMA, n_pairs_d - q0)
                    tt = tt_pool.tile([128, PAIRS_PER_DMA * 2 * TPAD], F8,
                                      tag=f"tt{tb}")
                    nc.gpsimd.dma_start(tt[:, 0:npr * 2 * TPAD],
                                        tT_ap[:, q0 * 2 * TPAD:(q0 + npr) * 2 * TPAD])
                    tt_tiles[tb] = tt
                return tt_tiles[tb]

            # A chunk tiles (fp8), resident for the whole execution
            a_tiles = [None] * len(n_chunks_cols)

            def get_a_chunk(ci):
                if a_tiles[ci] is None:
                    ncols = n_chunks_cols[ci]
                    cstart = sum(n_chunks_cols[:ci])
                    at = a_pool.tile([128, A_CHUNK], F8, tag=f"a{ci}")
                    nc.gpsimd.dma_start(at[:, 0:ncols], A_ap[:, cstart:cstart + ncols])
                    a_tiles[ci] = at
                return a_tiles[ci]

            # tgt control points broadcast [128, TGTF] (resident, f16)
            tgt_bc = fin_pool.tile([128, TGTF], F16, tag="tgtbc")

            def emit_tgt_chunk(i, n_chunks=4):
                c0 = i * (TGTF // n_chunks)
                c1 = (i + 1) * (TGTF // n_chunks)
                nc.gpsimd.dma_start(tgt_bc[:, c0:c1], tgtbc_ap[:, c0:c1])

            n_groups = len(group_meta)

            # ------------- per-batch state (reset each batch) -------------
            state = {}

            def emit_kpts(nb):
                b0, b1 = nbs[nb]
                nbW = b1 - b0
                ppk = state["ppk"]
                d_t = fin_pool.tile([128, TGTF], F16, tag="dkp", bufs=2)
                nc.vector.tensor_tensor(
                    out=d_t[0:nbW, :].rearrange("p (t k) -> p t k", k=KDIM),
                    in0=tgt_bc[0:nbW, :].rearrange("p (t k) -> p t k", k=KDIM),
                    in1=ppk[0:nbW, nb * KDIM:(nb + 1) * KDIM]
                        .broadcast_to([nbW, KDIM, T]).rearrange("p k t -> p t k"),
                    op=ALU.subtract)
                kp_t = fin_pool.tile([128, T], F32, tag=f"kp{nb}", bufs=2)
                nc.vector.tensor_reduce(
                    out=kp_t[0:nbW, :],
                    in_=d_t[0:nbW, :].rearrange("p (t k) -> p t k", k=KDIM),
                    axis=AX.X, op=ALU.add, apply_absolute_value=True)
                state[f"kp{nb}"] = kp_t

            # class-cost preludes for ALL batches, hoisted to program start and
            # grouped by activation function so the ACT table is switched a
            # constant number of times (steady state then runs Sigmoid only).
            prelude = {}

            def emit_preludes():
                cshape = [128, n_nb]

                def ptile(name, b):
                    return fin_pool.tile(cshape, F32, tag=f"{name}{b}",
                                         name=f"{name}{b}")

                for b in range(bpc):
                    lp16 = const_pool.tile([128, LPW], F16, tag=f"lp16_{b}",
                                           name=f"lp16_{b}")
                    src = masks_ap[:, b * SLAB + MROW:b * SLAB + MROW + 2 * LPW]
                    nc.gpsimd.dma_start(lp16[:], src.bitcast(F16))
                    prelude[f"lp16_{b}"] = lp16
                for b in range(bpc):
                    p_t = ptile("p", b)
                    nc.scalar.activation(p_t[:], prelude[f"lp16_{b}"][:, 0:n_nb],
                                         AF.Sigmoid)
                    prelude[f"p{b}"] = p_t
                for b in range(bpc):
                    q_t = ptile("q", b)
                    nc.vector.tensor_scalar(out=q_t[:], in0=prelude[f"p{b}"][:],
                                            scalar1=-1.0, scalar2=1.0,
                                            op0=ALU.mult, op1=ALU.add)
                    prelude[f"q{b}"] = q_t
                for b in range(bpc):
                    for nm in ("p", "q"):
                        ln_t = ptile(f"ln{nm}", b)
                        nc.scalar.activation(ln_t[:], prelude[f"{nm}{b}"][:],
                                             AF.Ln, bias=eps_col[:])
                        prelude[f"ln{nm}{b}"] = ln_t
                for b in range(bpc):
                    for nm in ("p", "q"):
                        sq_t = ptile(f"{nm}2", b)
                        nc.scalar.activation(sq_t[:], prelude[f"{nm}{b}"][:],
                                             AF.Square)
                        prelude[f"{nm}2{b}"] = sq_t
                for b in range(bpc):
                    t1_t = ptile("t1", b)
                    nc.vector.tensor_tensor(out=t1_t[:], in0=prelude[f"q2{b}"][:],
                                            in1=prelude[f"lnp{b}"][:], op=ALU.mult)
                    t2_t = ptile("t2", b)
                    nc.vector.tensor_tensor(out=t2_t[:], in0=prelude[f"p2{b}"][:],
                                            in1=prelude[f"lnq{b}"][:], op=ALU.mult)
                    t1s_t = ptile("t1s", b)
                    nc.vector.tensor_scalar(out=t1s_t[:], in0=t1_t[:],
                                            scalar1=-float(ALPHA) * CLASS_W,
                                            scalar2=5.0, op0=ALU.mult, op1=ALU.add)
                    base_t = ptile("base", b)
                    nc.vector.scalar_tensor_tensor(
                        out=base_t[:], in0=t2_t[:],
                        scalar=(1.0 - float(ALPHA)) * CLASS_W,
                        in1=t1s_t[:], op0=ALU.mult, op1=ALU.add)
                    prelude[f"base{b}"] = base_t

            pair_state = {}
            pending_nt = []

            def emit_pending_nt(keep=0):
                while len(pending_nt) > keep:
                    q, pu, st, tt, nsl = pending_nt.pop(0)
                    nc.scalar.activation(
                        st[:, 0:nsl * 304].rearrange("p (j n) -> p j n", n=304)[:, :, 0:n_rows],
                        pu[:, 0:nsl * 512].rearrange("p (j n) -> p j n", n=512)[:, :, 0:n_rows],
                        AF.Sigmoid)
                    tofs2 = (q % 4) * 2 * TPAD
                    st2 = st[:, 0:608].rearrange("p (two n) -> p two n", two=2)
                    tt2 = (tt[:, tofs2:tofs2 + 2 * TPAD]
                           .rearrange("p (two n) -> p two n", two=2))
                    if nsl == 2:
                        # DoubleRow: both groups of the pair as the 2 k-tiles
                        for nb, (b0, b1) in enumerate(nbs):
                            nc.tensor.matmul(
                                nt_slice(nb)[0:b1 - b0, 0:TPAD],
                                lhsT=st2[:, :, b0:b1],
                                rhs=tt2[:, :, 0:TPAD],
                                start=(q == 0),
                                stop=(2 * q + 1 == n_groups_d - 1),
                                perf_mode=mybir.MatmulPerfMode.DoubleRow,
                            )
                    else:
                        for nb, (b0, b1) in enumerate(nbs):
                            nc.tensor.matmul(
                                nt_slice(nb)[0:b1 - b0, 0:TPAD],
                                lhsT=st[:, b0:b1],
                                rhs=tt[:, tofs2:tofs2 + TPAD],
                                start=(q == 0),
                                stop=(2 * q == n_groups_d - 1),
                            )

            # mask pixel-rows r and r+1 are in the same DMA tile unless r+1
            # starts a new chunk
            bstarts = set([4] + list(range(16, n_rows_pix, ROWS_PER_DMA)))

            def emit_group(gi, mt_refs):
                p0, Wg, tiles = group_meta[gi]
                q, kk = gi // 2, gi % 2
                tb = q // 4
                tt = get_tt_batch(tb)
                if kk == 0:
                    pu = pu_pool.tile([128, 1024], F32, tag="pu")
                    st = st_pool.tile([128, 608], F8, tag="st")
                    pair_state[q] = (pu, st)
                else:
                    pu, st = pair_state.pop(q)
                pofs = kk * 512
                ci0 = tiles[0][1]
                at = get_a_chunk(ci0)
                if ci0 + 1 < len(n_chunks_cols):
                    get_a_chunk(ci0 + 1)  # prefetch
                wpad = (Wg + 15) // 16 * 16
                use_dr = (len(tiles) == 2
                          and tiles[1][0] == tiles[0][0] + 1
                          and tiles[1][2] == tiles[0][2] + wpad
                          and tiles[1][1] == ci0
                          and tiles[1][0] not in bstarts)
                if use_dr:
                    row, _, ofs, _ = tiles[0]
                    mt, j = mt_refs[row]
                    a2 = (at[:, ofs:ofs + 2 * wpad]
                          .rearrange("p (two w) -> p two w", two=2)[:, :, 0:Wg])
                    m2 = (mt[:, j * 304:(j + 2) * 304]
                          .rearrange("p (two n) -> p two n", two=2))
                    nc.tensor.matmul(
                        pu[0:Wg, pofs:pofs + n_rows],
                        lhsT=a2,
                        rhs=m2[:, :, 0:n_rows],
                        start=True, stop=True,
                        perf_mode=mybir.MatmulPerfMode.DoubleRow,
                    )
                else:
                    n_t = len(tiles)
                    for k, (row, ci, ofs, ncols) in enumerate(tiles):
                        at_k = get_a_chunk(ci)
                        mt, j = mt_refs[row]
                        nc.tensor.matmul(
                            pu[0:Wg, pofs:pofs + n_rows],
                            lhsT=at_k[:, ofs:ofs + ncols],
                            rhs=mt[:, j * 304:j * 304 + n_rows],
                            start=(k == 0),
                            stop=(k == n_t - 1),
                        )
                lone = (kk == 0 and gi == n_groups_d - 1)
                if kk == 1 or lone:
                    pending_nt.append((q, pu, st, tt, 1 if lone else 2))
                    emit_pending_nt(keep=2)

            def finalize_batch(b):
                for nb, (b0, b1) in enumerate(nbs):
                    nbW = b1 - b0
                    nts = nt_slice(nb)
                    # s_sum + 1  (on DVE so the ACT engine stays on Sigmoid)
                    ssum1 = fin_pool.tile([128, 1], F32, tag=f"ssum{nb}")
                    nc.vector.tensor_scalar(out=ssum1[0:nbW, :],
                                            in0=nts[0:nbW, 160:161],
                                            scalar1=1.0, scalar2=None,
                                            op0=ALU.add)
                    kp_t = state[f"kp{nb}"]
                    # den1 = t_sum_bcast + (s_sum + 1)
                    den1 = fin_pool.tile([128, T], F32, tag=f"den{nb}")
                    nc.vector.tensor_scalar(out=den1[0:nbW, :], in0=pden[0:nbW, 0:T],
                                            scalar1=ssum1[0:nbW, 0:1], scalar2=None,
                                            op0=ALU.add)
                    inv = fin_pool.tile([128, T], F32, tag=f"inv{nb}")
                    nc.vector.reciprocal(inv[0:nbW, :], den1[0:nbW, :])
                    # w = (2*num + 1) * inv
                    q2n = fin_pool.tile([128, T], F32, tag=f"q2n{nb}")
                    nc.vector.tensor_scalar(out=q2n[0:nbW, :], in0=nts[0:nbW, 0:T],
                                            scalar1=2.0, scalar2=1.0,
                                            op0=ALU.mult, op1=ALU.add)
                    w_t = fin_pool.tile([128, T], F32, tag=f"w{nb}")
                    nc.vector.tensor_tensor(out=w_t[0:nbW, :], in0=q2n[0:nbW, :],
                                            in1=inv[0:nbW, :], op=ALU.mult)
                    # C = 5*(kpts - w) + base
                    km_t = fin_pool.tile([128, T], F32, tag=f"km{nb}")
                    nc.vector.tensor_tensor(out=km_t[0:nbW, :], in0=kp_t[0:nbW, :],
                                            in1=w_t[0:nbW, :], op=ALU.subtract)
                    c_t = fin_pool.tile([128, T], F32, tag=f"c{nb}", bufs=2)
                    nc.vector.tensor_scalar(out=c_t[0:nbW, :], in0=km_t[0:nbW, :],
                                            scalar1=float(COORD_W),
                                            scalar2=prelude[f"base{b}"][0:nbW, nb:nb + 1],
                                            op0=ALU.mult, op1=ALU.add)
                    nc.sync.dma_start(C_ap[b * n_rows + b0:b * n_rows + b1, :],
                                      c_t[0:nbW, 0:T])

            # ---------------- per-batch mask pipeline ----------------
            # zero both pu rotation buffers once: PSUM holds stale data from
            # whatever ran before (possibly NaN), and partitions beyond a
            # group's Wg flow through sigmoid into the numerator matmul where
            # 0*NaN would poison the accumulation.  After this, every value
            # that ever lands in pu is finite.
            for _ in range(3):
                pz = pu_pool.tile([128, 1024], F32, tag="pu", name="pz")
                nc.vector.memset(pz[:], 0.0)
            get_a_chunk(0)
            get_tt_batch(0)
            emit_preludes()
            NTC = 8
            for b in range(bpc):
                mt_refs = {}
                g_idx = 0
                mofs = b * SLAB
                state["ppk"] = prelude[f"lp16_{b}"][:, n_nb:LPW]
                batch_starts = [0, 4] + list(range(16, n_rows_pix, ROWS_PER_DMA))
                if b == 0:
                    stage_at = {
                        16: lambda: (emit_tgt_chunk(0, NTC),
                                     emit_tgt_chunk(1, NTC), emit_tgt_chunk(2, NTC)),
                        32: lambda: (emit_tgt_chunk(3, NTC), emit_tgt_chunk(4, NTC),
                                     emit_tgt_chunk(5, NTC)),
                        48: lambda: (emit_tgt_chunk(6, NTC), emit_tgt_chunk(7, NTC)),
                        64: lambda: (emit_kpts(0), emit_kpts(1), emit_kpts(2)),
                    }
                else:
                    stage_at = {
                        64: lambda: (emit_kpts(0), emit_kpts(1), emit_kpts(2)),
                    }
                for bi, rb in enumerate(batch_starts):
                    fn = stage_at.get(rb)
                    if fn is not None:
                        fn()
                    nxt = batch_starts[bi + 1] if bi + 1 < len(batch_starts) else n_rows_pix
                    nr = nxt - rb
                    mt = mt_pool.tile([128, ROWS_PER_DMA * 304], F8, tag="mt")
                    nc.sync.dma_start(
                        mt[:, 0:nr * 304],
                        masks_ap[:, mofs + rb * 304:mofs + (rb + nr) * 304]
                        .bitcast(F8))
                    for j in range(nr):
                        r = rb + j
                        mt_refs[r] = (mt, j)
                        while g_idx < n_groups and group_meta[g_idx][2][-1][0] <= r:
                            emit_group(g_idx, mt_refs)
                            g_idx += 1
                while g_idx < n_groups:
                    emit_group(g_idx, mt_refs)
                    g_idx += 1
                emit_pending_nt()
                finalize_batch(b)

    nc.compile()
    return nc


def _get_runner(sig, shapes, consts, bpc, cores):
    ent = _CACHE.get(sig)
    if ent is not None:
        return ent
    import jax
    from jax.sharding import Mesh, PartitionSpec, NamedSharding
    from concourse import mybir
    from concourse.bass2jax import _bass_exec_p, install_neuronx_cc_hook

    install_neuronx_cc_hook()
    nc = _build_program(sig, shapes, consts, bpc)

    in_names, out_names, out_avals, zero_outs = [], [], [], []
    in_shapes = {}
    for alloc in nc.m.functions[0].allocations:
        if not isinstance(alloc, mybir.MemoryLocationSet):
            continue
        name = alloc.memorylocations[0].name
        if alloc.kind == "ExternalInput":
            in_names.append(name)
            in_shapes[name] = (tuple(alloc.tensor_shape), mybir.dt.np(alloc.dtype))
        elif alloc.kind == "ExternalOutput":
            shape = tuple(alloc.tensor_shape)
            dtype = mybir.dt.np(alloc.dtype)
            out_names.append(name)
            out_avals.append(jax.core.ShapedArray(shape, dtype))
            zero_outs.append(np.zeros(shape, dtype))

    all_in = tuple(in_names) + tuple(out_names)

    def _body(*args):
        return tuple(_bass_exec_p.bind(
            *args, out_avals=tuple(out_avals), in_names=all_in,
            out_names=tuple(out_names), lowering_input_output_aliases=(),
            sim_require_finite=False, sim_require_nnan=False, nc=nc))

    devs = jax.devices()[:cores]
    mesh = Mesh(np.asarray(devs), ("core",))
    nargs = len(in_names) + len(out_names)
    sm_kwargs = dict(mesh=mesh,
                     in_specs=(PartitionSpec("core"),) * nargs,
                     out_specs=(PartitionSpec("core"),) * len(out_names))
    sharding = NamedSharding(mesh, PartitionSpec("core"))

    def _make_shard_mapped():
        try:
            from jax import shard_map as _sm  # jax >= 0.8
            return _sm(_body, check_vma=False, **sm_kwargs)
        except (ImportError, TypeError):
            from jax.experimental.shard_map import shard_map as _sm_old
            return _sm_old(_body, check_rep=False, **sm_kwargs)

    # global (pre-shard_map) avals for AOT lowering
    shaped = []
    for name in in_names:
        shp, dt = in_shapes[name]
        shaped.append(jax.ShapeDtypeStruct((cores * shp[0],) + shp[1:], dt,
                                           sharding=sharding))
    for z in zero_outs:
        shaped.append(jax.ShapeDtypeStruct((cores * z.shape[0],) + z.shape[1:],
                                           z.dtype, sharding=sharding))

    sharded = None
    try:
        from concourse.bass2jax import fast_dispatch_compile

        def compile_fn():
            jitted = jax.jit(_make_shard_mapped(), keep_unused=True)
            return jitted.lower(*shaped).compile()

        sharded = fast_dispatch_compile(compile_fn)
        try:
            # drop the per-call safety-net shard walk (pure-python overhead on
            # every dispatch); our callers always read the outputs, so device
            # errors still surface at block_until_ready
            import jax._src.stages as _jstages
            sharded.__class__ = _jstages.Compiled
        except Exception:
            pass
    except Exception:
        import traceback
        traceback.print_exc()
        sharded = jax.jit(_make_shard_mapped(), keep_unused=True)

    ent = (sharded, in_names, out_names, zero_outs, sharding)
    _CACHE[sig] = ent
    return ent


# ----------------------------------------------------------------------------
# host fallback (lvl >= 2, or if the device path is unavailable)
# ----------------------------------------------------------------------------

def _host_reference(pred_logits, pred_ctrl_points, pred_seg_mask, tgt_pts,
                    tgt_masks, point_coords, lvl):
    bs, Q = pred_logits.shape[:2]
    N = bs * Q
    p = 1.0 / (1.0 + np.exp(-pred_logits.reshape(N, -1).astype(np.float64)))
    out_pts = pred_ctrl_points.reshape(N, -1).astype(np.float64)
    tgt_flat = tgt_pts.reshape(tgt_pts.shape[0], -1).astype(np.float64)

    cost_mask_dice = 0.0
    if int(lvl) < 2:
        H, W = pred_seg_mask.shape[-2:]
        x0, y0, wx0, wx1, wy0, wy1 = _point_grid(point_coords, H, W)
        t_samp = _point_sample_np(tgt_masks.astype(np.float32), x0, y0,
                                  wx0, wx1, wy0, wy1, H, W).astype(np.float64)
        o_masks = pred_seg_mask.reshape(N, H, W).astype(np.float32)
        o_samp = _point_sample_np(o_masks, x0, y0, wx0, wx1, wy0, wy1, H, W)
        s = 1.0 / (1.0 + np.exp(-o_samp.astype(np.float64)))
        num = 2.0 * (s @ t_samp.T)
        den = s.sum(-1)[:, None] + t_samp.sum(-1)[None, :]
        cost_mask_dice = 1.0 - (num + 1.0) / (den + 1.0)

    neg = (1 - 0.25) * p ** 2.0 * (-np.log(1.0 - p + EPS))
    pos = 0.25 * (1.0 - p) ** 2.0 * (-np.log(p + EPS))
    cost_class = pos - neg
    cost_kpts = np.abs(out_pts[:, None, :] - tgt_flat[None, :, :]).sum(-1)
    C = CLASS_W * cost_class + COORD_W * cost_kpts + MASK_W * cost_mask_dice
    return C.reshape(bs, Q, -1).astype(np.float32)


# ----------------------------------------------------------------------------
# main entry
# ----------------------------------------------------------------------------

_PREP_CACHE = {}


def _prepare(pred_logits, pred_ctrl_points, pred_seg_mask, tgt_pts, tgt_masks,
             point_coords):
    import ml_dtypes

    bs, Q = pred_logits.shape[:2]
    H, W = pred_seg_mask.shape[-2:]
    T = tgt_masks.shape[0]
    P = point_coords.shape[0]
    n_ctrl = pred_ctrl_points.shape[2]
    KDIM = 2 * n_ctrl

    key = hashlib.sha1()
    key.update(np.ascontiguousarray(point_coords).tobytes())
    key.update(np.ascontiguousarray(tgt_pts).tobytes())
    key.update(np.ascontiguousarray(tgt_masks).tobytes())
    key.update(str((bs, Q, H, W, T, P, n_ctrl, CORES)).encode())
    pkey = key.hexdigest()
    ent = _PREP_CACHE.get(pkey)
    if ent is not None:
        return ent

    x0, y0, wx0, wx1, wy0, wy1 = _point_grid(point_coords, H, W)
    perm, groups, A_packed, chunks = _build_groups(x0, y0, wx0, wx1, wy0, wy1, H, W)

    # target-side samples at the same points, in sorted-point order
    t_samp = _point_sample_np(tgt_masks.astype(np.float32), x0, y0,
                              wx0, wx1, wy0, wy1, H, W)      # [T, P] f32
    t_sum = t_samp.sum(axis=1, dtype=np.float32)             # [T]
    tTs = t_samp.T[perm].astype(np.float16)                  # [P, T] sorted
    # pair-interleaved packing: pair q = groups (2q, 2q+1); 128-row block q
    # holds [groupA TPAD cols | groupB TPAD cols] so DMA runs are 2*TPAD wide
    n_groups = len(groups)
    n_pairs = (n_groups + 1) // 2
    tT = np.zeros((n_pairs * 128, 2 * TPAD), np.float16)
    for gi, g in enumerate(groups):
        q, k = gi // 2, gi % 2
        r0 = q * 128
        sl = tT[r0:r0 + g["W"], k * TPAD:k * TPAD + T]
        sl[:] = tTs[g["p0"]:g["p0"] + g["W"]]
        tT[r0:r0 + g["W"], k * TPAD + T] = 1.0  # ones col -> s row-sum
    # partition-major repack: row p holds pair q's 2*TPAD block at q*2*TPAD
    tT = np.ascontiguousarray(
        tT.reshape(n_pairs, 128, 2 * TPAD).transpose(1, 0, 2)
          .reshape(128, n_pairs * 2 * TPAD))

    tgt_bc = np.ascontiguousarray(
        np.broadcast_to(tgt_pts.reshape(1, T * KDIM).astype(np.float16),
                        (128, T * KDIM)))

    consts = {
        "A": A_packed.astype(ml_dtypes.float8_e4m3),
        "tT": tT.astype(ml_dtypes.float8_e4m3),
        "tsum": t_sum.reshape(1, T).astype(np.float32),
        "tgtbc": tgt_bc,
    }

    group_meta = tuple(
        (g["p0"], g["W"], tuple((t[0], t[2], t[3], t[1].shape[1]) for t in g["tiles"]))
        for g in groups
    )
    sig = (bs, Q, H, W, T, P, n_ctrl, tuple(chunks), pkey)
    shapes = (Q, H * W, A_packed.shape[1], tuple(chunks), group_meta, T, P, n_ctrl)
    ent = (sig, shapes, consts)
    _PREP_CACHE[pkey] = ent
    return ent


def _make_in_maps(pred_logits, pred_ctrl_points, pred_seg_mask, Q, n_ctrl):
    """Per-core input dicts: masksT (fp8) in partition-major layout —
    masksT[x, b*SLAB + y*304 + q] = mask[b, q, y, x] for each of the core's
    BPC batch elements b, each slab followed by the packed logits|pts block
    (f16 values as byte pairs, bitcast on device)."""
    import ml_dtypes

    bs = pred_logits.shape[0]
    bpc = bs // CORES
    KDIM = 2 * n_ctrl
    n_nb = (Q + 127) // 128
    Hm, Wm = pred_seg_mask.shape[-2:]
    LPW = n_nb * (1 + KDIM)
    MROW = Hm * 304
    SLAB = MROW + 2 * LPW
    in_maps = []
    for c in range(CORES):
        mext8 = np.zeros((Wm, bpc * SLAB), ml_dtypes.float8_e4m3)
        mext = mext8.view(np.uint8)
        for j in range(bpc):
            b = c * bpc + j
            lp = np.zeros((128, LPW), np.float32)
            lg = pred_logits[b].reshape(Q)
            pc = pred_ctrl_points[b].reshape(Q, KDIM)
            for nb in range(n_nb):
                b0, b1 = nb * 128, min(nb * 128 + 128, Q)
                lp[0:b1 - b0, nb] = lg[b0:b1]
                lp[0:b1 - b0, n_nb + nb * KDIM:n_nb + (nb + 1) * KDIM] = pc[b0:b1]
            m3 = np.zeros((Wm, Hm, 304), ml_dtypes.float8_e4m3)
            m3[:, :, 0:Q] = pred_seg_mask[b].transpose(2, 1, 0).astype(
                ml_dtypes.float8_e4m3)
            mext8[:, j * SLAB:j * SLAB + MROW] = m3.reshape(Wm, MROW)
            mext[:, j * SLAB + MROW:(j + 1) * SLAB] = (
                lp.astype(np.float16).view(np.uint8))
        in_maps.append({"masksT": mext})
    return in_maps


def kernel(pred_logits, pred_ctrl_points, pred_seg_mask, tgt_pts, tgt_masks,
           point_coords, lvl):
    pred_logits = np.asarray(pred_logits)
    pred_ctrl_points = np.asarray(pred_ctrl_points)
    pred_seg_mask = np.asarray(pred_seg_mask)
    tgt_pts = np.asarray(tgt_pts)
    tgt_masks = np.asarray(tgt_masks)
    point_coords = np.asarray(point_coords)

    if int(lvl) >= 2:
        return _host_reference(pred_logits, pred_ctrl_points, pred_seg_mask,
                               tgt_pts, tgt_masks, point_coords, lvl)

    try:
        return _device_kernel(pred_logits, pred_ctrl_points, pred_seg_mask,
                              tgt_pts, tgt_masks, point_coords)
    except Exception:
        import traceback
        traceback.print_exc()
        return _host_reference(pred_logits, pred_ctrl_points, pred_seg_mask,
                               tgt_pts, tgt_masks, point_coords, lvl)


def _device_kernel(pred_logits, pred_ctrl_points, pred_seg_mask, tgt_pts,
                   tgt_masks, point_coords):
    import jax

    bs, Q = pred_logits.shape[:2]
    T = tgt_masks.shape[0]
    n_ctrl = pred_ctrl_points.shape[2]

    sig, shapes, consts = _prepare(pred_logits, pred_ctrl_points, pred_seg_mask,
                                   tgt_pts, tgt_masks, point_coords)
    if bs % CORES != 0 or len(jax.devices()) < CORES:
        raise RuntimeError(f"need bs % {CORES} == 0 cores, got bs={bs}, "
                           f"{len(jax.devices())} devices")
    bpc = bs // CORES
    sharded, in_names, out_names, zero_outs, sharding = _get_runner(
        sig, shapes, consts, bpc, CORES)
    in_maps = _make_in_maps(pred_logits, pred_ctrl_points, pred_seg_mask,
                            Q, n_ctrl)

    gargs = [
        jax.device_put(
            np.concatenate([in_maps[c][n] for c in range(CORES)], axis=0), sharding)
        for n in in_names
    ]
    gargs += [
        jax.device_put(
            np.zeros((CORES * z.shape[0], *z.shape[1:]), z.dtype), sharding)
        for z in zero_outs
    ]
    outs = sharded(*gargs)
    C = np.asarray(outs[0]).reshape(bs, Q, T)
    return C.astype(np.float32)
